# revision 45
# baseline (speedup 1.0000x reference)
"""Causal self-attention TRN2 kernel (8 NeuronCores, Megatron-style sharding).

Reference computation (fp32):
    qkv = x @ w_attn.T ; q,k,v split; per-head causal softmax(q k^T/sqrt(hs)) v
    out = y @ w_proj.T
Shapes: x [4, 2048, 1024], w_attn [3072, 1024], w_proj [1024, 1024], 16 heads.

Sharding: core = (b, g) with b = batch 0..3, g = head-group 0..1 (8 heads each).
Each core computes its batch's attention for its 8 heads plus the partial
output projection over its 512 local head-dims; host sums the two partials
per batch (Megatron row-parallel) and transposes back.

Device dataflow is fully transposed ([feature, token] layout) so the PE
contraction dim always sits on partitions with zero on-device transposes:
  qkT[d, t] = waT.T @ xT           (lhsT = waT block, rhs = xT)
  V[t, d]   = xT.T @ waT_v         (lhsT = xT block, rhs = wv)
  S.T[k, q] = KT.T @ QT            (the two heads of a pair run CONCURRENTLY
                                    on PE row groups 0-1 / 2-3 and write the
                                    two halves of a [128,1024] PSUM)
  P = exp(S/8) on the scalar engine (values bounded, no max-subtraction);
      softmax denominators ride free as a ones column appended to V (the y.T
      matmul has M=65, row 64 = sum_k P)
  y.T[d, q] = V_aug.T @ P          (accumulated over k-tiles in PSUM)
  outT[e, q] = wpT.T @ yT          (partial over local d)

Schedule: the exp stream on the scalar engine (ACT) is the pace-setter of
the attention phase (~1.1us per k-tile vs ~0.76us of PE work), so the
projection matmuls (qkT d-tiles, V t-tiles, out-proj e-tiles) are
interleaved INTO the attention k-tile loop as "fillers" right before the
q-chunk that first needs them.  All weights/activations are host-packed
into a handful of contiguous DRAM tensors so input staging is ~15 large
DMAs instead of ~110 small ones (DMA issue on the sync queue costs ~0.6us
each).  Matmuls run in bf16 (fp32 PSUM accumulation); softmax sums and
reciprocals in fp32 (reciprocal_approx_fast); output written bf16 (host
accumulates the two partial sums per batch in fp32).
"""

import math

import numpy as np

import concourse.bass as bass
import concourse.tile as tile
from concourse import bacc, mybir
from concourse import bass_utils

F32 = mybir.dt.float32
BF16 = mybir.dt.bfloat16
DT = BF16

C = 1024          # embed dim
NH_LOCAL = 8      # heads per core
HS = 64           # head size
DL = NH_LOCAL * HS  # local head-dim total (512)
NCT = C // 128    # c-tiles (contraction tiles) = 8


def build(T: int = 2048):
    """Build + compile the per-core program for sequence length T."""
    NQC = T // 512    # q-chunks
    NKT = T // 128    # k-tiles / t-tiles
    NE = C // 128     # output-projection e-tiles

    nc = bacc.Bacc(
        "TRN2", target_bir_lowering=False, debug=False, enable_asserts=False
    )

    # host-packed layouts (see kernel() for the exact packing)
    xB = nc.dram_tensor("xB", [128, NCT * T], DT, kind="ExternalInput").ap()
    waB = nc.dram_tensor("waB", [128, 8 * 1024], DT, kind="ExternalInput").ap()
    wvB = nc.dram_tensor("wvB", [128, NCT * DL], DT, kind="ExternalInput").ap()
    wpB = nc.dram_tensor("wpB", [128, 4 * C], DT, kind="ExternalInput").ap()
    tri2 = nc.dram_tensor("tri2", [128, 256], DT, kind="ExternalInput").ap()
    outT = nc.dram_tensor("outT", [C, T], DT, kind="ExternalOutput").ap()

    with tile.TileContext(nc) as tc:
        with (
            tc.tile_pool(name="const", bufs=1) as constp,
            tc.tile_pool(name="persist", bufs=1) as persist,
            tc.tile_pool(name="stage", bufs=4) as stagep,
            tc.tile_pool(name="epool", bufs=6) as epool,
            tc.tile_pool(name="sumsp", bufs=2) as sumsp,
            tc.tile_pool(name="misc", bufs=3) as miscp,
            tc.tile_pool(name="rbp", bufs=10) as rbp,
            tc.tile_pool(name="yup", bufs=18) as yup,
            tc.tile_pool(name="ps_small", bufs=2, space="PSUM") as ps_small,
            tc.tile_pool(name="ps_st", bufs=2, space="PSUM") as ps_st,
            tc.tile_pool(name="ps_yt", bufs=2, space="PSUM") as ps_yt,
        ):
            # ---- constants (DMA'd after the startup-critical tensors) ----
            tri_t = constp.tile([128, 256], DT, tag="tri", name="tri_t")
            tri3 = tri_t[:].rearrange("p (h q) -> p h q", h=2)

            # ---- persistent activations ----
            va_t = []  # V augmented with ones column: [128, 8*65]
            for tt in range(NKT):
                va = persist.tile(
                    [128, NH_LOCAL * (HS + 1)], DT, tag=f"va{tt}", name=f"va{tt}"
                )
                va_t.append(va)
            yt_t = []  # y.T per head-pair: [128, T]
            for p in range(4):
                yt = persist.tile([128, T], DT, tag=f"yt{p}", name=f"yt{p}")
                yt_t.append(yt)
            qk_t = []  # qkT resident: tiles 0-3 = QT pairs, 4-7 = KT pairs
            for dt in range(8):
                qk = persist.tile([128, T], DT, tag=f"qk{dt}", name=f"qk{dt}")
                qk_t.append(qk)

            # ============ DMA staging (need-ordered, host-packed) ============
            DT_ORDER = [0, 4, 1, 5, 2, 6, 3, 7]
            wa_t = {}
            xt = persist.tile([128, NCT * T], DT, tag="xt", name="xt")
            wa = persist.tile([128, 1024], DT, tag="wa0", name="wa0")
            nc.sync.dma_start(wa[:], waB[:, 0:1024])
            wa_t[0] = wa
            # first x chunk split in two so the transfers run on two queues
            nc.sync.dma_start(xt[:, 0 : NCT * 256], xB[:, 0 : NCT * 256])
            nc.sync.dma_start(
                xt[:, NCT * 256 : NCT * 512], xB[:, NCT * 256 : NCT * 512]
            )
            wa = persist.tile([128, 1024], DT, tag="wa4", name="wa4")
            nc.sync.dma_start(wa[:], waB[:, 4096:5120])
            wa_t[4] = wa
            wv = persist.tile([128, NCT * DL], DT, tag="wv", name="wv")
            nc.sync.dma_start(wv[:], wvB[:])
            nc.sync.dma_start(tri_t[:], tri2[:])
            for jq in range(1, NQC):
                nc.sync.dma_start(
                    xt[:, NCT * 512 * jq : NCT * 512 * (jq + 1)],
                    xB[:, NCT * 512 * jq : NCT * 512 * (jq + 1)],
                )
            for dt in DT_ORDER[2:]:
                wa = persist.tile([128, 1024], DT, tag=f"wa{dt}", name=f"wa{dt}")
                nc.sync.dma_start(wa[:], waB[:, 1024 * dt : 1024 * (dt + 1)])
                wa_t[dt] = wa
            wp = persist.tile([128, 4 * C], DT, tag="wp", name="wp")
            nc.sync.dma_start(wp[:], wpB[:])

            # ============ filler chunks (projection matmuls) ============
            def a_chunk(dt, jq, engine="vector"):
                """qkT[dt][:, jq-chunk] = waT_dt.T @ xT  (8 MMs + drain)."""
                ps = ps_small.tile([128, 512], F32, tag="psA", name="psA")
                for ci in range(NCT):
                    nc.tensor.matmul(
                        ps[:],
                        wa_t[dt][:, 128 * ci : 128 * (ci + 1)],
                        xt[:, 4096 * jq + 512 * ci : 4096 * jq + 512 * (ci + 1)],
                        start=(ci == 0),
                        stop=(ci == NCT - 1),
                    )
                dst = qk_t[dt][:, 512 * jq : 512 * (jq + 1)]
                if engine == "scalar":
                    nc.scalar.copy(dst, ps[:])
                else:
                    nc.vector.tensor_copy(dst, ps[:])

            def v_chunk(tt, engine="vector"):
                """va[tt] = xT.T @ wv (+ ones column per head)."""
                ps = ps_small.tile([128, 512], F32, tag="psA", name="psV")
                jq, o = tt // 4, 128 * (tt % 4)
                for ci in range(NCT):
                    c0 = 4096 * jq + 512 * ci + o
                    nc.tensor.matmul(
                        ps[:],
                        xt[:, c0 : c0 + 128],
                        wv[:, 512 * ci : 512 * (ci + 1)],
                        start=(ci == 0),
                        stop=(ci == NCT - 1),
                    )
                va = va_t[tt]
                va3 = va[:].rearrange("p (h d) -> p h d", d=HS + 1)
                ps3 = ps[:].rearrange("p (h d) -> p h d", d=HS)
                if engine == "scalar":
                    nc.scalar.copy(va3[:, :, 0:HS], ps3[:])
                else:
                    nc.vector.tensor_copy(va3[:, :, 0:HS], ps3[:])
                nc.vector.memset(va3[:, :, HS].bitcast(mybir.dt.uint16), 0x3F80)

            def c_chunk(jq, e):
                """outT[e-tile, jq-chunk] = wpT.T @ yT (4 MMs + drain + DMA).

                Drains on the scalar engine: phase C runs during/after the
                last q-chunk where ACT has slack, and this keeps the DVE
                free for the softmax-normalization chain.
                """
                ps = ps_small.tile([128, 512], F32, tag="psA", name="psC")
                for p4 in range(4):
                    nc.tensor.matmul(
                        ps[:],
                        wp[:, 128 * (4 * e + p4) : 128 * (4 * e + p4 + 1)],
                        yt_t[p4][:, 512 * jq : 512 * (jq + 1)],
                        start=(p4 == 0),
                        stop=(p4 == 3),
                    )
                ot = stagep.tile([128, 512], DT, tag="stage", name="stC")
                nc.scalar.copy(ot[:], ps[:])
                nc.sync.dma_start(
                    outT[128 * e : 128 * (e + 1), 512 * jq : 512 * (jq + 1)],
                    ot[:],
                )

            # ============ startup: minimum needed before B(p0, j0) ============
            a_chunk(0, 0, engine="scalar")
            a_chunk(4, 0, engine="scalar")
            for tt in range(4):
                v_chunk(tt, engine="scalar")

            # ============ phase B: attention with interleaved fillers ========
            EXPF = mybir.ActivationFunctionType.Exp
            ISCALE = 1.0 / math.sqrt(HS)
            pending_norm = []

            for p in range(4):  # head pairs
                qt, kt = qk_t[p], qk_t[4 + p]
                # (j, h) softmax sums parked at 32-aligned partitions (engine
                # APs require 32-aligned partition bases) so one reciprocal
                # covers 4 units at once
                NS = (2 * NQC + 3) // 4
                sums = [
                    sumsp.tile([128, 512], F32, tag=f"sums{s}", name=f"sums{s}")
                    for s in range(NS)
                ]
                rcs = [
                    sumsp.tile([128, 512], F32, tag=f"rcs{s}", name=f"rcs{s}")
                    for s in range(NS)
                ]
                for s in range(NS):
                    nc.gpsimd.memset(sums[s][:], 1.0)
                yus = {}

                def _recip_s(s, half=None, sums=sums, rcs=rcs):
                    # denominators are positive and well within fp32 range;
                    # ~18 correct bits is far beyond what bf16 yT keeps
                    rows = (
                        slice(None)
                        if half is None
                        else slice(64 * half, 64 * (half + 1))
                    )
                    nc.vector.reciprocal_approx_fast(
                        rcs[s][rows, :], sums[s][rows, :]
                    )

                rbs = {}

                def _norm_prep(j, h, rcs=rcs, rbs=rbs):
                    # stage the reciprocal row at partition 0 (the HW
                    # broadcast reads partition 0 only) and broadcast it;
                    # done well ahead of the multiply so the gpsimd burst
                    # doesn't collide with the DVE drain burst
                    r = 2 * j + h
                    r0 = miscp.tile([1, 512], DT, tag="r0", name="r0")
                    nc.vector.tensor_copy(
                        r0[:], rcs[r // 4][32 * (r % 4) : 32 * (r % 4) + 1, :]
                    )
                    rb = rbp.tile([64, 512], DT, tag="rb", name="rb")
                    nc.gpsimd.partition_broadcast(rb[:], r0[:])
                    rbs[(j, h)] = rb

                def _norm_mul(j, h, p=p, yus=yus, rbs=rbs):
                    qs = slice(512 * j, 512 * (j + 1))
                    nc.vector.tensor_mul(
                        yt_t[p][64 * h : 64 * (h + 1), qs],
                        yus.pop((j, h))[:],
                        rbs.pop((j, h))[:],
                    )


                def _norm_one(j, h):
                    _norm_prep(j, h)
                    _norm_mul(j, h)

                for j in range(NQC):
                    # fillers feeding the NEXT consumer of qkT/V data
                    fillers = []
                    if p < 3 or j < NQC - 1:
                        nxt = (j + 1) % NQC
                        tgt = p if j < NQC - 1 else p + 1
                        # fillers feeding the next PAIR drain on the scalar
                        # engine (its boundary slack); mid-pair ones stay on
                        # the DVE so the exp pacer isn't delayed
                        eng = "vector" if j < NQC - 1 else "scalar"
                        fillers.append(
                            lambda d=tgt, q=nxt, g=eng: a_chunk(d, q, engine=g)
                        )
                        fillers.append(
                            lambda d=tgt + 4, q=nxt, g=eng: a_chunk(d, q, engine=g)
                        )
                    if p == 0 and j < NQC - 1:
                        for tt in range(4 * (j + 1), 4 * (j + 2)):
                            fillers.append(lambda t=tt: v_chunk(t))
                    # deferred normalization units from the previous pair
                    if j >= 1 and pending_norm:
                        take = (
                            6 if j < NQC - 1 else len(pending_norm)
                        )
                        fillers.extend(pending_norm[:take])
                        pending_norm = pending_norm[take:]
                    if p == 3 and j == NQC - 2:
                        # sums rows for j0/j1 are final: start the reciprocal
                        fillers.append(lambda f=_recip_s: f(0))
                    if p == 3 and j == NQC - 1:
                        # normalize j0..j2 and run phase C for those q-chunks
                        # inside the last (longest) k-tile loop; j2's sums
                        # rows live in the lower half of the second tile and
                        # are final once j2 ended
                        for jj in range(NQC - 2):
                            for h in range(2):
                                fillers.append(
                                    lambda a=jj, b=h, f=_norm_prep: f(a, b)
                                )
                        for jj in range(NQC - 2):
                            for h in range(2):
                                fillers.append(
                                    lambda a=jj, b=h, f=_norm_mul: f(a, b)
                                )
                        for jj in range(NQC - 2):
                            for e in range(NE):
                                fillers.append(
                                    lambda a=jj, b=e, f=c_chunk: f(a, b)
                                )
                        fillers.append(lambda f=_recip_s: f(1, half=0))
                        for h in range(2):
                            fillers.append(
                                lambda b=h, f=_norm_prep: f(NQC - 2, b)
                            )
                        for h in range(2):
                            fillers.append(
                                lambda b=h, f=_norm_mul: f(NQC - 2, b)
                            )
                        for e in range(NE):
                            fillers.append(
                                lambda b=e, f=c_chunk: f(NQC - 2, b)
                            )

                    n_kt = 4 * j + 4
                    LAG = 3
                    n_slots = n_kt + LAG
                    fill_at = {}
                    if fillers:
                        for fi, fn in enumerate(fillers):
                            slot = min(
                                n_slots - 1, (fi * n_slots) // len(fillers)
                            )
                            fill_at.setdefault(slot, []).append(fn)

                    qs = slice(512 * j, 512 * (j + 1))
                    ytps = [
                        ps_yt.tile([HS + 1, 512], F32, tag="ytp", name="ytp0"),
                        ps_yt.tile([HS + 1, 512], F32, tag="ytp", name="ytp1"),
                    ]
                    ets = {}
                    for i in range(n_slots):
                        for fn in fill_at.get(i, ()):
                            fn()
                        if i < n_kt:
                            ks = slice(128 * i, 128 * (i + 1))
                            o = 128 * (i - 4 * j)  # diag block offset, <0 if past
                            op = max(o, 0)
                            # both heads' S.T into one [128,1024] PSUM; skip
                            # the known-invalid q-prefix of diag tiles
                            st = ps_st.tile([128, 1024], F32, tag="stp", name="stp")
                            for h in range(2):
                                hp = slice(64 * h, 64 * (h + 1))
                                nc.tensor.matmul(
                                    st[:, 512 * h + op : 512 * (h + 1)],
                                    kt[hp, ks],
                                    qt[hp, 512 * j + op : 512 * (j + 1)],
                                    start=True,
                                    stop=True,
                                )
                            et = epool.tile([128, 1024], DT, tag="et", name="et")
                            if o <= 128:
                                # fully-causal tile (or cheap single call)
                                nc.scalar.activation(
                                    et[:], st[:], EXPF, scale=ISCALE
                                )
                            else:  # o in {256, 384}: split beats one call
                                for h in range(2):
                                    c0 = 512 * h
                                    nc.scalar.activation(
                                        et[:, c0 + o : c0 + 512],
                                        st[:, c0 + o : c0 + 512],
                                        EXPF,
                                        scale=ISCALE,
                                    )
                            if o >= 0:  # mask diagonal block, both heads at once
                                # on gpsimd: the mask gates the next y.T, and
                                # the DVE queue at pair boundaries would delay
                                # it by several us
                                et3 = et[:].rearrange("p (h q) -> p h q", h=2)
                                nc.gpsimd.tensor_mul(
                                    et3[:, :, o : o + 128],
                                    et3[:, :, o : o + 128],
                                    tri3,
                                )
                            ets[i] = (et, op)
                        ic = i - LAG  # consume earlier k-tile
                        if ic >= 0:
                            et, op = ets.pop(ic)
                            for h in range(2):
                                hh = 2 * p + h
                                nc.tensor.matmul(
                                    ytps[h][:, op:512],
                                    va_t[ic][:, 65 * hh : 65 * hh + 65],
                                    et[:, 512 * h + op : 512 * (h + 1)],
                                    start=(ic == 0),
                                    stop=(ic == n_kt - 1),
                                )
                    for h in range(2):
                        # park the fp32 sum row straight from PSUM first (it
                        # heads the reciprocal chain), then drain the rest of
                        # the accumulator to bf16
                        r = 2 * j + h
                        nc.vector.tensor_copy(
                            sums[r // 4][32 * (r % 4) : 32 * (r % 4) + 1, :],
                            ytps[h][HS : HS + 1, :],
                        )
                        yu = yup.tile([HS, 512], DT, tag="yu", name="yu")
                        nc.vector.tensor_copy(yu[:], ytps[h][0:HS, :])
                        yus[(j, h)] = yu

                # Normalization, deferred into the next pair's schedule (the
                # last pair normalizes inline, interleaved with phase C).
                if p < 3:
                    pending_norm.extend(
                        [lambda s=s, f=_recip_s: f(s) for s in range(NS)]
                        + [
                            (lambda j=j, h=h, f=_norm_prep: f(j, h))
                            for j in range(NQC)
                            for h in range(2)
                        ]
                        + [
                            (lambda j=j, h=h, f=_norm_mul: f(j, h))
                            for j in range(NQC)
                            for h in range(2)
                        ]
                    )
                else:
                    # tail: only the last q-chunk's normalization + phase C.
                    # Full-tile reciprocal: a base-partition-64 slice of the
                    # custom DVE op produced wrong values on HW, so recompute
                    # rows 0-63 redundantly (same cost; the op is FD-paced).
                    _recip_s(1)
                    for h in range(2):
                        _norm_prep(NQC - 1, h)
                    for h in range(2):
                        _norm_mul(NQC - 1, h)
                    for e in range(NE):
                        c_chunk(NQC - 1, e)

    nc.compile()
    return nc


_CACHE: dict = {}
_LAST_IN_MAPS = None


def _get_nc(T: int):
    if T not in _CACHE:
        _CACHE[T] = build(T)
    return _CACHE[T]


def kernel(x: np.ndarray, w_attn: np.ndarray, w_proj: np.ndarray) -> np.ndarray:
    import ml_dtypes

    B, T, C_ = x.shape
    nc = _get_nc(T)
    bf = ml_dtypes.bfloat16
    kk = np.arange(128)[:, None]
    cc = np.arange(128)[None, :]
    tri = (cc >= kk).astype(bf)
    tri2 = np.concatenate([tri, tri], axis=1)

    in_maps = []
    for core in range(8):
        b, g = core // 2, core % 2
        heads = range(8 * g, 8 * g + 8)
        rows = []
        for base in (0, C_, 2 * C_):  # q, k, v sections of w_attn
            for H in heads:
                rows.extend(range(base + 64 * H, base + 64 * H + 64))
        waT_l = np.ascontiguousarray(np.asarray(w_attn)[rows, :].T).astype(bf)
        dcols = [c for H in heads for c in range(64 * H, 64 * H + 64)]
        wpT_l = np.ascontiguousarray(np.asarray(w_proj)[:, dcols].T).astype(bf)
        xT_l = np.ascontiguousarray(np.asarray(x[b]).T).astype(bf)

        # pack into partition-major contiguous layouts (one DMA per block):
        #   waB[p, 1024*dt + 128*ci + c] = waT_l[128*ci + p, 128*dt + c]
        #   wvB[p, 512*ci + c]           = waT_l[128*ci + p, 1024 + c]
        #   xB [p, 4096*jq + 512*ci + c] = xT_l[128*ci + p, 512*jq + c]
        #   wpB[p, 128*(4*e + p4) + c]   = wpT_l[128*p4 + p, 128*e + c]
        tmp = waT_l.reshape(8, 128, 1536)
        waB = np.ascontiguousarray(
            tmp[:, :, :1024].reshape(8, 128, 8, 128).transpose(1, 2, 0, 3)
        ).reshape(128, 8192)
        wvB = np.ascontiguousarray(tmp[:, :, 1024:].transpose(1, 0, 2)).reshape(
            128, 4096
        )
        xB = np.ascontiguousarray(
            xT_l.reshape(8, 128, T // 512, 512).transpose(1, 2, 0, 3)
        ).reshape(128, 8 * T)
        wpB = np.ascontiguousarray(
            wpT_l.reshape(4, 128, 8, 128).transpose(1, 2, 0, 3)
        ).reshape(128, 4096)
        in_maps.append(
            {"xB": xB, "waB": waB, "wvB": wvB, "wpB": wpB, "tri2": tri2}
        )

    global _LAST_IN_MAPS
    _LAST_IN_MAPS = in_maps
    res = bass_utils.run_bass_kernel_spmd(nc, in_maps, core_ids=list(range(8)))
    out = np.empty((B, T, C_), dtype=np.float32)
    for b in range(B):
        out[b] = (
            res.results[2 * b]["outT"].astype(np.float32)
            + res.results[2 * b + 1]["outT"].astype(np.float32)
        ).T
    return out


# revision 46
# speedup vs baseline: 1.2130x; 1.2130x over previous
"""Causal self-attention TRN2 kernel (8 NeuronCores, Megatron-style sharding).

Reference computation (fp32):
    qkv = x @ w_attn.T ; q,k,v split; per-head causal softmax(q k^T/sqrt(hs)) v
    out = y @ w_proj.T
Shapes: x [4, 2048, 1024], w_attn [3072, 1024], w_proj [1024, 1024], 16 heads.

Sharding: core = (b, g) with b = batch 0..3, g = head-group 0..1 (8 heads each).
Each core computes its batch's attention for its 8 heads plus the partial
output projection over its 512 local head-dims; host sums the two partials
per batch (Megatron row-parallel) and transposes back.

Device dataflow is fully transposed ([feature, token] layout) so the PE
contraction dim always sits on partitions with zero on-device transposes:
  qkT[d, t] = waT.T @ xT           (lhsT = waT block, rhs = xT)
  V[t, d]   = xT.T @ waT_v         (lhsT = xT block, rhs = wv)
  S.T[k, q] = KT.T @ QT            (the two heads of a pair run CONCURRENTLY
                                    on PE row groups 0-1 / 2-3 and write the
                                    two halves of a [128,1024] PSUM)
  P = exp(S/8) on the scalar engine (values bounded, no max-subtraction);
      softmax denominators ride free as a ones column appended to V (the y.T
      matmul has M=65, row 64 = sum_k P)
  y.T[d, q] = V_aug.T @ P          (accumulated over k-tiles in PSUM)
  outT[e, q] = wpT.T @ yT          (partial over local d)

Schedule: the exp stream on the scalar engine (ACT) is the pace-setter of
the attention phase (~1.1us per k-tile vs ~0.76us of PE work), so the
projection matmuls (qkT d-tiles, V t-tiles, out-proj e-tiles) are
interleaved INTO the attention k-tile loop as "fillers" right before the
q-chunk that first needs them.  All weights/activations are host-packed
into a handful of contiguous DRAM tensors so input staging is ~15 large
DMAs instead of ~110 small ones (DMA issue on the sync queue costs ~0.6us
each).  Matmuls run in bf16 (fp32 PSUM accumulation); softmax sums and
reciprocals in fp32 (reciprocal_approx_fast); output written bf16 (host
accumulates the two partial sums per batch in fp32).
"""

import math

import numpy as np

import concourse.bass as bass
import concourse.tile as tile
from concourse import bacc, mybir
from concourse import bass_utils

F32 = mybir.dt.float32
BF16 = mybir.dt.bfloat16
DT = BF16

C = 1024          # embed dim
NH_LOCAL = 8      # heads per core
HS = 64           # head size
DL = NH_LOCAL * HS  # local head-dim total (512)
NCT = C // 128    # c-tiles (contraction tiles) = 8


def build(T: int = 2048):
    """Build + compile the per-core program for sequence length T."""
    NQC = T // 512    # q-chunks
    NKT = T // 128    # k-tiles / t-tiles
    NE = C // 128     # output-projection e-tiles

    nc = bacc.Bacc(
        "TRN2", target_bir_lowering=False, debug=False, enable_asserts=False
    )

    # host-packed layouts (see kernel() for the exact packing)
    xB = nc.dram_tensor("xB", [128, NCT * T], DT, kind="ExternalInput").ap()
    waB = nc.dram_tensor("waB", [128, 8 * 1024], DT, kind="ExternalInput").ap()
    wvB = nc.dram_tensor("wvB", [128, NCT * DL], DT, kind="ExternalInput").ap()
    wpB = nc.dram_tensor("wpB", [128, 4 * C], DT, kind="ExternalInput").ap()
    tri2 = nc.dram_tensor("tri2", [128, 256], DT, kind="ExternalInput").ap()
    outT = nc.dram_tensor("outT", [C, T], DT, kind="ExternalOutput").ap()

    with tile.TileContext(nc) as tc:
        with (
            tc.tile_pool(name="const", bufs=1) as constp,
            tc.tile_pool(name="persist", bufs=1) as persist,
            tc.tile_pool(name="stage", bufs=4) as stagep,
            tc.tile_pool(name="epool", bufs=6) as epool,
            tc.tile_pool(name="sumsp", bufs=2) as sumsp,
            tc.tile_pool(name="misc", bufs=3) as miscp,
            tc.tile_pool(name="rbp", bufs=10) as rbp,
            tc.tile_pool(name="yup", bufs=18) as yup,
            tc.tile_pool(name="ps_small", bufs=2, space="PSUM") as ps_small,
            tc.tile_pool(name="ps_st", bufs=2, space="PSUM") as ps_st,
            tc.tile_pool(name="ps_yt", bufs=2, space="PSUM") as ps_yt,
        ):
            # ---- constants (DMA'd after the startup-critical tensors) ----
            tri_t = constp.tile([128, 256], DT, tag="tri", name="tri_t")
            tri3 = tri_t[:].rearrange("p (h q) -> p h q", h=2)

            # ---- persistent activations ----
            va_t = []  # V augmented with ones column: [128, 8*65]
            for tt in range(NKT):
                va = persist.tile(
                    [128, NH_LOCAL * (HS + 1)], DT, tag=f"va{tt}", name=f"va{tt}"
                )
                va_t.append(va)
            yt_t = []  # y.T per head-pair: [128, T]
            for p in range(4):
                yt = persist.tile([128, T], DT, tag=f"yt{p}", name=f"yt{p}")
                yt_t.append(yt)
            qk_t = []  # qkT resident: tiles 0-3 = QT pairs, 4-7 = KT pairs
            for dt in range(8):
                qk = persist.tile([128, T], DT, tag=f"qk{dt}", name=f"qk{dt}")
                qk_t.append(qk)

            # ============ DMA staging (need-ordered, host-packed) ============
            DT_ORDER = [0, 4, 1, 5, 2, 6, 3, 7]
            wa_t = {}
            xt = persist.tile([128, NCT * T], DT, tag="xt", name="xt")
            wa = persist.tile([128, 1024], DT, tag="wa0", name="wa0")
            nc.sync.dma_start(wa[:], waB[:, 0:1024])
            wa_t[0] = wa
            # first x chunk split in two so the transfers run on two queues
            nc.sync.dma_start(xt[:, 0 : NCT * 256], xB[:, 0 : NCT * 256])
            nc.sync.dma_start(
                xt[:, NCT * 256 : NCT * 512], xB[:, NCT * 256 : NCT * 512]
            )
            wa = persist.tile([128, 1024], DT, tag="wa4", name="wa4")
            nc.sync.dma_start(wa[:], waB[:, 4096:5120])
            wa_t[4] = wa
            wv = persist.tile([128, NCT * DL], DT, tag="wv", name="wv")
            nc.sync.dma_start(wv[:], wvB[:])
            nc.sync.dma_start(tri_t[:], tri2[:])
            for jq in range(1, NQC):
                nc.sync.dma_start(
                    xt[:, NCT * 512 * jq : NCT * 512 * (jq + 1)],
                    xB[:, NCT * 512 * jq : NCT * 512 * (jq + 1)],
                )
            for dt in DT_ORDER[2:]:
                wa = persist.tile([128, 1024], DT, tag=f"wa{dt}", name=f"wa{dt}")
                nc.sync.dma_start(wa[:], waB[:, 1024 * dt : 1024 * (dt + 1)])
                wa_t[dt] = wa
            wp = persist.tile([128, 4 * C], DT, tag="wp", name="wp")
            nc.sync.dma_start(wp[:], wpB[:])

            # ============ filler chunks (projection matmuls) ============
            def a_chunk(dt, jq, engine="vector"):
                """qkT[dt][:, jq-chunk] = waT_dt.T @ xT  (8 MMs + drain)."""
                ps = ps_small.tile([128, 512], F32, tag="psA", name="psA")
                for ci in range(NCT):
                    nc.tensor.matmul(
                        ps[:],
                        wa_t[dt][:, 128 * ci : 128 * (ci + 1)],
                        xt[:, 4096 * jq + 512 * ci : 4096 * jq + 512 * (ci + 1)],
                        start=(ci == 0),
                        stop=(ci == NCT - 1),
                    )
                dst = qk_t[dt][:, 512 * jq : 512 * (jq + 1)]
                if engine == "scalar":
                    nc.scalar.copy(dst, ps[:])
                else:
                    nc.vector.tensor_copy(dst, ps[:])

            def v_chunk(tt, engine="vector"):
                """va[tt] = xT.T @ wv (+ ones column per head)."""
                ps = ps_small.tile([128, 512], F32, tag="psA", name="psV")
                jq, o = tt // 4, 128 * (tt % 4)
                for ci in range(NCT):
                    c0 = 4096 * jq + 512 * ci + o
                    nc.tensor.matmul(
                        ps[:],
                        xt[:, c0 : c0 + 128],
                        wv[:, 512 * ci : 512 * (ci + 1)],
                        start=(ci == 0),
                        stop=(ci == NCT - 1),
                    )
                va = va_t[tt]
                va3 = va[:].rearrange("p (h d) -> p h d", d=HS + 1)
                ps3 = ps[:].rearrange("p (h d) -> p h d", d=HS)
                if engine == "scalar":
                    nc.scalar.copy(va3[:, :, 0:HS], ps3[:])
                else:
                    nc.vector.tensor_copy(va3[:, :, 0:HS], ps3[:])
                nc.vector.memset(va3[:, :, HS].bitcast(mybir.dt.uint16), 0x3F80)

            def c_chunk(jq, e):
                """outT[e-tile, jq-chunk] = wpT.T @ yT (4 MMs + drain + DMA).

                Drains on the scalar engine: phase C runs during/after the
                last q-chunk where ACT has slack, and this keeps the DVE
                free for the softmax-normalization chain.
                """
                ps = ps_small.tile([128, 512], F32, tag="psA", name="psC")
                for p4 in range(4):
                    nc.tensor.matmul(
                        ps[:],
                        wp[:, 128 * (4 * e + p4) : 128 * (4 * e + p4 + 1)],
                        yt_t[p4][:, 512 * jq : 512 * (jq + 1)],
                        start=(p4 == 0),
                        stop=(p4 == 3),
                    )
                ot = stagep.tile([128, 512], DT, tag="stage", name="stC")
                nc.scalar.copy(ot[:], ps[:])
                nc.sync.dma_start(
                    outT[128 * e : 128 * (e + 1), 512 * jq : 512 * (jq + 1)],
                    ot[:],
                )

            # ============ startup: minimum needed before B(p0, j0) ============
            a_chunk(0, 0, engine="scalar")
            a_chunk(4, 0, engine="scalar")
            for tt in range(4):
                v_chunk(tt, engine="scalar")

            # ============ phase B: attention with interleaved fillers ========
            EXPF = mybir.ActivationFunctionType.Exp
            ISCALE = 1.0 / math.sqrt(HS)
            pending_norm = []

            for p in range(4):  # head pairs
                qt, kt = qk_t[p], qk_t[4 + p]
                # (j, h) softmax sums parked at 32-aligned partitions (engine
                # APs require 32-aligned partition bases) so one reciprocal
                # covers 4 units at once
                NS = (2 * NQC + 3) // 4
                sums = [
                    sumsp.tile([128, 512], F32, tag=f"sums{s}", name=f"sums{s}")
                    for s in range(NS)
                ]
                rcs = [
                    sumsp.tile([128, 512], F32, tag=f"rcs{s}", name=f"rcs{s}")
                    for s in range(NS)
                ]
                for s in range(NS):
                    nc.gpsimd.memset(sums[s][:], 1.0)
                yus = {}

                def _recip_s(s, half=None, sums=sums, rcs=rcs):
                    # denominators are positive and well within fp32 range;
                    # ~18 correct bits is far beyond what bf16 yT keeps
                    rows = (
                        slice(None)
                        if half is None
                        else slice(64 * half, 64 * (half + 1))
                    )
                    nc.vector.reciprocal_approx_fast(
                        rcs[s][rows, :], sums[s][rows, :]
                    )

                rbs = {}

                def _norm_prep(j, h, rcs=rcs, rbs=rbs):
                    # stage the reciprocal row at partition 0 (the HW
                    # broadcast reads partition 0 only) and broadcast it;
                    # done well ahead of the multiply so the gpsimd burst
                    # doesn't collide with the DVE drain burst
                    r = 2 * j + h
                    r0 = miscp.tile([1, 512], DT, tag="r0", name="r0")
                    nc.vector.tensor_copy(
                        r0[:], rcs[r // 4][32 * (r % 4) : 32 * (r % 4) + 1, :]
                    )
                    rb = rbp.tile([64, 512], DT, tag="rb", name="rb")
                    nc.gpsimd.partition_broadcast(rb[:], r0[:])
                    rbs[(j, h)] = rb

                def _norm_mul(j, h, p=p, yus=yus, rbs=rbs):
                    qs = slice(512 * j, 512 * (j + 1))
                    nc.vector.tensor_mul(
                        yt_t[p][64 * h : 64 * (h + 1), qs],
                        yus.pop((j, h))[:],
                        rbs.pop((j, h))[:],
                    )


                def _norm_one(j, h):
                    _norm_prep(j, h)
                    _norm_mul(j, h)

                for j in range(NQC):
                    # fillers feeding the NEXT consumer of qkT/V data
                    fillers = []
                    if p < 3 or j < NQC - 1:
                        nxt = (j + 1) % NQC
                        tgt = p if j < NQC - 1 else p + 1
                        # fillers feeding the next PAIR drain on the scalar
                        # engine (its boundary slack); mid-pair ones stay on
                        # the DVE so the exp pacer isn't delayed
                        eng = "vector" if j < NQC - 1 else "scalar"
                        fillers.append(
                            lambda d=tgt, q=nxt, g=eng: a_chunk(d, q, engine=g)
                        )
                        fillers.append(
                            lambda d=tgt + 4, q=nxt, g=eng: a_chunk(d, q, engine=g)
                        )
                    if p == 0 and j < NQC - 1:
                        for tt in range(4 * (j + 1), 4 * (j + 2)):
                            fillers.append(lambda t=tt: v_chunk(t))
                    # deferred normalization units from the previous pair
                    if j >= 1 and pending_norm:
                        take = (
                            6 if j < NQC - 1 else len(pending_norm)
                        )
                        fillers.extend(pending_norm[:take])
                        pending_norm = pending_norm[take:]
                    if p == 3 and j == NQC - 2:
                        # sums rows for j0/j1 are final: start the reciprocal
                        fillers.append(lambda f=_recip_s: f(0))
                    if p == 3 and j == NQC - 1:
                        # normalize j0..j2 and run phase C for those q-chunks
                        # inside the last (longest) k-tile loop; j2's sums
                        # rows live in the lower half of the second tile and
                        # are final once j2 ended
                        for jj in range(NQC - 2):
                            for h in range(2):
                                fillers.append(
                                    lambda a=jj, b=h, f=_norm_prep: f(a, b)
                                )
                        for jj in range(NQC - 2):
                            for h in range(2):
                                fillers.append(
                                    lambda a=jj, b=h, f=_norm_mul: f(a, b)
                                )
                        for jj in range(NQC - 2):
                            for e in range(NE):
                                fillers.append(
                                    lambda a=jj, b=e, f=c_chunk: f(a, b)
                                )
                        fillers.append(lambda f=_recip_s: f(1, half=0))
                        for h in range(2):
                            fillers.append(
                                lambda b=h, f=_norm_prep: f(NQC - 2, b)
                            )
                        for h in range(2):
                            fillers.append(
                                lambda b=h, f=_norm_mul: f(NQC - 2, b)
                            )
                        for e in range(NE):
                            fillers.append(
                                lambda b=e, f=c_chunk: f(NQC - 2, b)
                            )

                    n_kt = 4 * j + 4
                    LAG = 3
                    n_slots = n_kt + LAG
                    fill_at = {}
                    if fillers:
                        for fi, fn in enumerate(fillers):
                            slot = min(
                                n_slots - 1, (fi * n_slots) // len(fillers)
                            )
                            fill_at.setdefault(slot, []).append(fn)

                    qs = slice(512 * j, 512 * (j + 1))
                    ytps = [
                        ps_yt.tile([HS + 1, 512], F32, tag="ytp", name="ytp0"),
                        ps_yt.tile([HS + 1, 512], F32, tag="ytp", name="ytp1"),
                    ]
                    ets = {}
                    for i in range(n_slots):
                        for fn in fill_at.get(i, ()):
                            fn()
                        if i < n_kt:
                            ks = slice(128 * i, 128 * (i + 1))
                            o = 128 * (i - 4 * j)  # diag block offset, <0 if past
                            op = max(o, 0)
                            # both heads' S.T into one [128,1024] PSUM; skip
                            # the known-invalid q-prefix of diag tiles
                            st = ps_st.tile([128, 1024], F32, tag="stp", name="stp")
                            for h in range(2):
                                hp = slice(64 * h, 64 * (h + 1))
                                nc.tensor.matmul(
                                    st[:, 512 * h + op : 512 * (h + 1)],
                                    kt[hp, ks],
                                    qt[hp, 512 * j + op : 512 * (j + 1)],
                                    start=True,
                                    stop=True,
                                )
                            et = epool.tile([128, 1024], DT, tag="et", name="et")
                            if o <= 128:
                                # fully-causal tile (or cheap single call)
                                nc.scalar.activation(
                                    et[:], st[:], EXPF, scale=ISCALE
                                )
                            else:  # o in {256, 384}: split beats one call
                                for h in range(2):
                                    c0 = 512 * h
                                    nc.scalar.activation(
                                        et[:, c0 + o : c0 + 512],
                                        st[:, c0 + o : c0 + 512],
                                        EXPF,
                                        scale=ISCALE,
                                    )
                            if o >= 0:  # mask diagonal block, both heads at once
                                et3 = et[:].rearrange("p (h q) -> p h q", h=2)
                                nc.vector.tensor_mul(
                                    et3[:, :, o : o + 128],
                                    et3[:, :, o : o + 128],
                                    tri3,
                                )
                            ets[i] = (et, op)
                        ic = i - LAG  # consume earlier k-tile
                        if ic >= 0:
                            et, op = ets.pop(ic)
                            for h in range(2):
                                hh = 2 * p + h
                                nc.tensor.matmul(
                                    ytps[h][:, op:512],
                                    va_t[ic][:, 65 * hh : 65 * hh + 65],
                                    et[:, 512 * h + op : 512 * (h + 1)],
                                    start=(ic == 0),
                                    stop=(ic == n_kt - 1),
                                )
                    for h in range(2):
                        # park the fp32 sum row straight from PSUM first (it
                        # heads the reciprocal chain), then drain the rest of
                        # the accumulator to bf16
                        r = 2 * j + h
                        nc.vector.tensor_copy(
                            sums[r // 4][32 * (r % 4) : 32 * (r % 4) + 1, :],
                            ytps[h][HS : HS + 1, :],
                        )
                        yu = yup.tile([HS, 512], DT, tag="yu", name="yu")
                        nc.vector.tensor_copy(yu[:], ytps[h][0:HS, :])
                        yus[(j, h)] = yu

                # Normalization, deferred into the next pair's schedule (the
                # last pair normalizes inline, interleaved with phase C).
                if p < 3:
                    pending_norm.extend(
                        [lambda s=s, f=_recip_s: f(s) for s in range(NS)]
                        + [
                            (lambda j=j, h=h, f=_norm_prep: f(j, h))
                            for j in range(NQC)
                            for h in range(2)
                        ]
                        + [
                            (lambda j=j, h=h, f=_norm_mul: f(j, h))
                            for j in range(NQC)
                            for h in range(2)
                        ]
                    )
                else:
                    # tail: only the last q-chunk's normalization + phase C.
                    # Full-tile reciprocal: a base-partition-64 slice of the
                    # custom DVE op produced wrong values on HW, so recompute
                    # rows 0-63 redundantly (same cost; the op is FD-paced).
                    _recip_s(1)
                    for h in range(2):
                        _norm_prep(NQC - 1, h)
                    for h in range(2):
                        _norm_mul(NQC - 1, h)
                    for e in range(NE):
                        c_chunk(NQC - 1, e)

    nc.compile()
    return nc


_CACHE: dict = {}
_LAST_IN_MAPS = None


def _get_nc(T: int):
    if T not in _CACHE:
        _CACHE[T] = build(T)
    return _CACHE[T]


def kernel(x: np.ndarray, w_attn: np.ndarray, w_proj: np.ndarray) -> np.ndarray:
    import ml_dtypes

    B, T, C_ = x.shape
    nc = _get_nc(T)
    bf = ml_dtypes.bfloat16
    kk = np.arange(128)[:, None]
    cc = np.arange(128)[None, :]
    tri = (cc >= kk).astype(bf)
    tri2 = np.concatenate([tri, tri], axis=1)

    in_maps = []
    for core in range(8):
        b, g = core // 2, core % 2
        heads = range(8 * g, 8 * g + 8)
        rows = []
        for base in (0, C_, 2 * C_):  # q, k, v sections of w_attn
            for H in heads:
                rows.extend(range(base + 64 * H, base + 64 * H + 64))
        waT_l = np.ascontiguousarray(np.asarray(w_attn)[rows, :].T).astype(bf)
        dcols = [c for H in heads for c in range(64 * H, 64 * H + 64)]
        wpT_l = np.ascontiguousarray(np.asarray(w_proj)[:, dcols].T).astype(bf)
        xT_l = np.ascontiguousarray(np.asarray(x[b]).T).astype(bf)

        # pack into partition-major contiguous layouts (one DMA per block):
        #   waB[p, 1024*dt + 128*ci + c] = waT_l[128*ci + p, 128*dt + c]
        #   wvB[p, 512*ci + c]           = waT_l[128*ci + p, 1024 + c]
        #   xB [p, 4096*jq + 512*ci + c] = xT_l[128*ci + p, 512*jq + c]
        #   wpB[p, 128*(4*e + p4) + c]   = wpT_l[128*p4 + p, 128*e + c]
        tmp = waT_l.reshape(8, 128, 1536)
        waB = np.ascontiguousarray(
            tmp[:, :, :1024].reshape(8, 128, 8, 128).transpose(1, 2, 0, 3)
        ).reshape(128, 8192)
        wvB = np.ascontiguousarray(tmp[:, :, 1024:].transpose(1, 0, 2)).reshape(
            128, 4096
        )
        xB = np.ascontiguousarray(
            xT_l.reshape(8, 128, T // 512, 512).transpose(1, 2, 0, 3)
        ).reshape(128, 8 * T)
        wpB = np.ascontiguousarray(
            wpT_l.reshape(4, 128, 8, 128).transpose(1, 2, 0, 3)
        ).reshape(128, 4096)
        in_maps.append(
            {"xB": xB, "waB": waB, "wvB": wvB, "wpB": wpB, "tri2": tri2}
        )

    global _LAST_IN_MAPS
    _LAST_IN_MAPS = in_maps
    res = bass_utils.run_bass_kernel_spmd(nc, in_maps, core_ids=list(range(8)))
    out = np.empty((B, T, C_), dtype=np.float32)
    for b in range(B):
        out[b] = (
            res.results[2 * b]["outT"].astype(np.float32)
            + res.results[2 * b + 1]["outT"].astype(np.float32)
        ).T
    return out
